# revision 25
# baseline (speedup 1.0000x reference)
"""Trainium2 Bass kernel for nn_MixedAttention.

Full inputs in, full output out. Sharding: 8 cores = 2 (batch) x 4 (head
pairs). Each core computes 2 global + 2 local heads for one batch element.

Key algebraic rewrite for the local branch:
    lscores = (lq@lk1^T)@(lk1@lk2^T) = lq @ (lk1^T@lk1) @ lk2^T
with M = lk1^T@lk1 a [64,64] matrix -- turns a 2048^3 matmul chain into
two small matmuls plus one S x S matmul (30x less PE work).

Dtype strategy: fp32 matmuls cost 4 cycles/row on the PE, float32r
(TF32-like, ~13-bit mantissa) costs 1 at free-size >= 512. Measured
rel-err budget is 2e-2 and a numerical simulation of 13-bit input
rounding through the local score chain lands at ~1.4e-3, so EVERY
matmul input here is f32r (storage is bit-identical to fp32; the PE
rounds internally). hidden/weights are declared f32r straight from
DRAM so no conversion copies exist anywhere.

Layout: scores are computed transposed st[j, i] = K_eff @ Q_eff^T so the
context matmul needs no transposed probs (lhsT = v_nat, rhs = e). v gets
an extra ones column so the softmax denominator falls out of the context
matmul for free. Global heads skip max subtraction entirely (mask folded
into the Exp bias); local heads get an exact row max from a pass-1 f32r
matmul sweep in the untransposed orientation (free-dim reduce_max), and
the -max correction rides an extra contraction row (K=65) in pass 2.

Schedule: phase A1 = input DMA (split fine-grained over 4 trigger
queues) + all 7 projections with 4-wide PSUM accumulation. Phase A2 =
global attention pair-units interleaved with local-prep and the pass-1
max sweep (hides pass-1's DVE reduces under global-attention PE work).
Phase B = local pass-2 attention. Attention works in ic-pairs: one
[128,1024] two-bank PSUM st tile per j-block, one Exp activation per
j-block, ctx accumulated in a [65,1024] two-bank PSUM tile.
"""

import math
import os
import sys

import numpy as np

sys.path.insert(0, "/opt/trn_rl_repo")

B, S, HID, HEAD = 2, 2048, 1024, 64
SC = S // 128  # 16 s-chunks of 128
HC = HID // 128  # 8 hidden chunks
N_CORES = 8
SCALE = 1.0 / math.sqrt(HEAD)

W_NAMES = ["wq", "wk", "wv", "wlq", "wlk1", "wlk2", "wlv"]

_CACHE = {}
LAST_RESULTS = None  # stash of BassKernelResults for test.py profiling


def _build():
    import concourse.mybir as mybir
    import concourse.tile as tile
    from concourse import bacc
    from concourse.masks import make_identity

    f32 = mybir.dt.float32
    f32r = mybir.dt.float32r
    bf16 = mybir.dt.bfloat16
    AF = mybir.ActivationFunctionType
    ALU = mybir.AluOpType
    AX = mybir.AxisListType

    nc = bacc.Bacc("TRN2", target_bir_lowering=False, debug=False,
                   enable_asserts=False)

    hid_d = nc.dram_tensor("hid", (HID, S), f32r, kind="ExternalInput").ap()
    mask_d = nc.dram_tensor("mask", (S,), f32, kind="ExternalInput").ap()
    w_d = {n: nc.dram_tensor(n, (HID, 128), f32r, kind="ExternalInput").ap()
           for n in W_NAMES}
    b_d = {n: nc.dram_tensor("b" + n[1:], (128,), f32,
                             kind="ExternalInput").ap() for n in W_NAMES}
    out_d = nc.dram_tensor("out", (S, 256), f32, kind="ExternalOutput").ap()

    dma_engines = None  # set inside the TileContext

    def dma_rr(i):
        return dma_engines[i % len(dma_engines)]

    with tile.TileContext(nc) as tc:
        dma_engines = [nc.sync, nc.gpsimd, nc.scalar]
        with (
            tc.tile_pool(name="const", bufs=1) as constp,
            tc.tile_pool(name="persist", bufs=1) as pp,
            tc.tile_pool(name="vgpool", bufs=1) as vgp,
            tc.tile_pool(name="ps_mm", bufs=3, space="PSUM") as ps_mm,
            tc.tile_pool(name="ps_ctx", bufs=1, space="PSUM") as ps_ctx,
            tc.tile_pool(name="dramp", bufs=2, space="DRAM") as dramp,
        ):
            ident = constp.tile([128, 128], f32, name="ident")
            make_identity(nc, ident)
            ones_sb = constp.tile([128, SC], f32, name="ones_sb")
            nc.vector.memset(ones_sb, 1.0)
            ones_row = constp.tile([1, S], f32, name="ones_row")
            nc.vector.memset(ones_row, 1.0)
            mask_sb = constp.tile([128, SC], f32, name="mask_sb")
            nc.gpsimd.dma_start(mask_sb,
                                mask_d.rearrange("(c p) -> p c", p=128))
            bias_sb = {}
            for n in W_NAMES:
                t = constp.tile([128, 1], f32, name=f"b_{n}")
                nc.gpsimd.dma_start(t, b_d[n][:, None])
                bias_sb[n] = t

            # local-branch projections persist into phase B
            projT = {n: pp.tile([128, S],
                                f32 if n in ("wlk1", "wlv") else f32r,
                                name=f"projT_{n}")
                     for n in ["wlq", "wlk1", "wlk2", "wlv"]}

            # assigned when the A2/B pools open (the helpers below only
            # run after that)
            vp = ep = op_ = out_sb = None

            # ---------- emission helpers ----------

            def build_vaug(vT):
                # v natural [s, d] + ones column -> [128, SC, 65] bf16
                # (bf16 halves the per-ctx-matmul LDWEIGHTS stream; value
                # path tolerates the 8-bit mantissa)
                base = vT.base_partition()
                idsl = slice(base, base + 64)
                vaug = vgp.tile([128, SC, 65], bf16, tag="vaug",
                                name="vaug", bufs=4)
                nc.vector.tensor_copy(vaug[:, :, 64], ones_sb)
                for t in range(SC):
                    pt = ps_mm.tile([128, 1024], f32, tag="mm", name="ptv")
                    nc.tensor.transpose(
                        pt[:, :64], vT[:, t * 128:(t + 1) * 128],
                        ident[idsl, idsl])
                    nc.vector.tensor_copy(vaug[:, t, :64], pt[:, :64])
                return vaug

            def attention_pair(head, kT, qT, vaug, is_local, jp,
                               filler=None, defer_tail=False,
                               copy_on_act=True):
                # pair-unit: 2 i-column blocks of 512; st -> exp -> ctx
                # (+denominators via the ones column), then transpose back
                # and divide by the sums. `filler` emits ACT-independent PE
                # work each jc so the tensor engine never starves while the
                # Exp runs (starvation resets the PE's DVFS ramp).
                csl = slice(head * 64, (head + 1) * 64)
                i0 = jp * 1024
                ctx = ps_ctx.tile([65, 1024], f32, tag="ctx", name="ctx")

                def ctx_mm(jc, e):
                    nc.tensor.matmul(ctx[:, 0:512], lhsT=vaug[:, jc],
                                     rhs=e[:, 0:512],
                                     start=(jc == 0), stop=(jc == SC - 1))
                    nc.tensor.matmul(ctx[:, 512:1024], lhsT=vaug[:, jc],
                                     rhs=e[:, 512:1024],
                                     start=(jc == 0), stop=(jc == SC - 1))

                # one-jc software pipeline lag: the ctx matmuls for jc are
                # emitted after st/exp of jc+1, so the in-order PE queue
                # never waits on the Exp of the tile it just produced
                prev = None
                for jc in range(SC):
                    jsl = slice(jc * 128, (jc + 1) * 128)
                    st = ps_mm.tile([128, 1024], f32, tag="mm", name="st")
                    nc.tensor.matmul(st[:, 0:512], lhsT=kT[:, jsl],
                                     rhs=qT[:, i0:i0 + 512],
                                     start=True, stop=True)
                    nc.tensor.matmul(st[:, 512:1024], lhsT=kT[:, jsl],
                                     rhs=qT[:, i0 + 512:i0 + 1024],
                                     start=True, stop=True)
                    e = ep.tile([128, 1024], bf16, tag="e", name="e")
                    bias = 0.0 if is_local else mask_sb[:, jc:jc + 1]
                    nc.scalar.activation(e, st, AF.Exp, bias=bias,
                                         scale=SCALE)
                    if filler is not None:
                        filler()
                    if prev is not None:
                        ctx_mm(*prev)
                    prev = (jc, e)
                ctx_mm(*prev)
                ctx_sbc = vp.tile([65, 1024], f32, tag="ctx_sbc",
                                  name="ctx_sbc", bufs=2)
                # the PSUM->SBUF ctx copy rides ACT when DVE is the busier
                # engine (phase A2) and DVE when ACT paces the phase (B)
                if copy_on_act:
                    nc.scalar.copy(ctx_sbc, ctx)
                else:
                    nc.vector.tensor_copy(ctx_sbc, ctx)

                def make_tg(tg):
                    def run():
                        pts = ps_mm.tile([128, 1024], f32, tag="mm",
                                         name="pts")
                        rec = vp.tile([128, 4], f32, tag="rec", name="rec",
                                      bufs=2)
                        for q in range(4):
                            tt = tg * 4 + q
                            nc.tensor.transpose(
                                pts[:, q * 256:q * 256 + 65],
                                ctx_sbc[:, tt * 128:(tt + 1) * 128],
                                ident[:65, :65])
                            nc.vector.reciprocal(
                                rec[:, q:q + 1],
                                pts[:, q * 256 + 64:q * 256 + 65])
                        for q in range(4):
                            tt = tg * 4 + q
                            t_abs = jp * 8 + tt
                            nc.vector.tensor_scalar_mul(
                                out_sb[:, t_abs, csl],
                                pts[:, q * 256:q * 256 + 64],
                                rec[:, q:q + 1])
                        nc.gpsimd.dma_start(
                            out_d.rearrange("(t p) c -> p t c", p=128)[
                                :, jp * 8 + tg * 4:jp * 8 + tg * 4 + 4, csl],
                            out_sb[:, jp * 8 + tg * 4:jp * 8 + tg * 4 + 4,
                                   csl])
                    return run

                tail = [make_tg(0), make_tg(1)]
                if defer_tail:
                    return tail
                for th in tail:
                    th()
                return []

            def local_prep(head):
                hh = head % 2
                rs = slice(hh * 64, (hh + 1) * 64)
                idsl = slice(rs.start, rs.start + 64)
                if hh == 0:
                    lqT = projT["wlq"][rs]
                else:
                    # matmul operands must share a base partition and the
                    # PSUM dst must sit at partition 0, so head hh=1's lq
                    # is staged down to base 0 (ACT handles the shift)
                    lqT = vp.tile([64, S], f32r, tag="lqT", name="lqT",
                                  bufs=1)
                    nc.scalar.copy(lqT, projT["wlq"][rs])
                lk1T = projT["wlk1"][rs]

                # lk1 natural [s, d] via transposes (identity block at the
                # source base partition avoids any staging copy)
                lk1nat = vp.tile([128, SC, 64], f32r, tag="lk1nat",
                                 name="lk1nat", bufs=2)
                for t in range(SC):
                    pt = ps_mm.tile([128, 1024], f32, tag="mm", name="ptk")
                    nc.tensor.transpose(
                        pt[:, :64], lk1T[:, t * 128:(t + 1) * 128],
                        ident[idsl, idsl])
                    nc.vector.tensor_copy(lk1nat[:, t], pt[:, :64])
                # M = lk1^T @ lk1 [64, 64] (symmetric)
                mps = ps_mm.tile([128, 1024], f32, tag="mm", name="mps")
                for t in range(SC):
                    nc.tensor.matmul(mps[:64, :64], lhsT=lk1nat[:, t],
                                     rhs=lk1nat[:, t],
                                     start=(t == 0), stop=(t == SC - 1))
                m_sb = vp.tile([64, 64], f32r, tag="m_sb", name="m_sb",
                               bufs=2)
                nc.vector.tensor_copy(m_sb, mps[:64, :64])
                # qaug rows 0:64 = (lq @ M)^T = M @ lq^T (M symmetric);
                # row 64 filled later with -max
                qaug = vp.tile([65, S], f32r, tag="qaug", name="qaug",
                               bufs=2)
                for half in range(2):
                    mm = ps_mm.tile([128, 1024], f32, tag="mm", name="mm")
                    for ic in range(2):
                        icg = half * 2 + ic
                        nc.tensor.matmul(
                            mm[:64, ic * 512:(ic + 1) * 512], lhsT=m_sb,
                            rhs=lqT[:, icg * 512:(icg + 1) * 512],
                            start=True, stop=True)
                    nc.vector.tensor_copy(
                        qaug[:64, half * 1024:(half + 1) * 1024], mm[:64])
                # k2aug: rows 0:64 = lk2^T, row 64 = ones
                k2aug = vp.tile([65, S], f32r, tag="k2aug", name="k2aug",
                                bufs=2)
                nc.scalar.copy(k2aug[:64, :], projT["wlk2"][rs])
                nc.vector.tensor_copy(k2aug[64:65, :], ones_row)
                vaug = build_vaug(projT["wlv"][rs])
                pmax = vp.tile([128, SC, 2], f32, tag="pmax", name="pmax",
                               bufs=2)
                return dict(qaug=qaug, k2aug=k2aug, vaug=vaug, pmax=pmax)

            def pass1_pair(hs, t, jp):
                # one (t, jp) unit of the pass-1 max sweep: raw scores in
                # the untransposed orientation, row max via free-dim reduce
                qaug, k2aug, pmax = hs["qaug"], hs["k2aug"], hs["pmax"]
                tsl = slice(t * 128, (t + 1) * 128)
                st = ps_mm.tile([128, 1024], f32, tag="mm", name="st1")
                for j2 in range(2):
                    j0 = jp * 1024 + j2 * 512
                    nc.tensor.matmul(st[:, j2 * 512:(j2 + 1) * 512],
                                     lhsT=qaug[:64, tsl],
                                     rhs=k2aug[:64, j0:j0 + 512],
                                     start=True, stop=True)
                nc.vector.tensor_reduce(pmax[:, t, jp:jp + 1], st,
                                        axis=AX.X, op=ALU.max)

            def make_filler(units):
                it = iter(units)

                def filler():
                    u = next(it, None)
                    if u is not None:
                        pass1_pair(*u)
                return filler

            def pass1_finish(hs):
                # combine pair maxes, negate, and route [128, SC] -> [1, S]
                # via a DRAM roundtrip into qaug row 64
                maxneg = vp.tile([128, SC], f32r, tag="maxneg",
                                 name="maxneg", bufs=2)
                nc.vector.tensor_reduce(maxneg, hs["pmax"], axis=AX.X,
                                        op=ALU.max, negate=True)
                mscr = dramp.tile([S], f32r, tag="mscr", name="mscr")
                nc.sync.dma_start(
                    mscr.rearrange("(t p) -> p t", p=128), maxneg)
                nc.sync.dma_start(hs["qaug"][64:65, :], mscr[None, :])

            # ---------- phase A1: hidden^T + all 7 projections ----------
            with tc.tile_pool(name="pp_g", bufs=1) as pp_g:
                for n in ["wq", "wk", "wv"]:
                    projT[n] = pp_g.tile([128, S],
                                         f32 if n == "wv" else f32r,
                                         name=f"projT_{n}")

                with (
                    tc.tile_pool(name="hidT", bufs=1) as hp,
                    tc.tile_pool(name="io", bufs=4) as iop,
                ):
                    hidT = hp.tile([128, HC, S], f32r, name="hidT")
                    hid_r = hid_d.rearrange("(c p) s -> p c s", p=128)
                    dmai = 0
                    wsbs = {}

                    def emit_wdma(n):
                        nonlocal dmai
                        wsb = iop.tile([128, HC, 128], f32r, tag="w",
                                       name=f"w_{n}")
                        w_r = w_d[n].rearrange("(c p) m -> p c m", p=128)
                        for h in range(2):
                            dma_rr(dmai).dma_start(wsb[:, h * 4:h * 4 + 4],
                                                   w_r[:, h * 4:h * 4 + 4])
                            dmai += 1
                        wsbs[n] = wsb

                    def emit_hid_chunk(hc):
                        nonlocal dmai
                        for icq in range(4):
                            ssl = slice(icq * 512, (icq + 1) * 512)
                            dma_rr(dmai).dma_start(hidT[:, hc, ssl],
                                                   hid_r[:, hc, ssl])
                            dmai += 1

                    # interleave so proj(wq) can start the moment its
                    # weight halves + hid chunk 0 land
                    emit_wdma("wq")
                    emit_hid_chunk(0)
                    emit_wdma("wk")
                    emit_hid_chunk(1)
                    emit_wdma("wv")
                    for hc in range(2, HC):
                        emit_hid_chunk(hc)

                    def emit_proj(n):
                        accs = [ps_mm.tile([128, 1024], f32, tag="mm",
                                           name=f"acc{i}") for i in range(2)]
                        for hc in range(HC):
                            for ic in range(4):
                                nc.tensor.matmul(
                                    accs[ic // 2][:, (ic % 2) * 512:
                                                  (ic % 2 + 1) * 512],
                                    lhsT=wsbs[n][:, hc],
                                    rhs=hidT[:, hc, ic * 512:(ic + 1) * 512],
                                    start=(hc == 0), stop=(hc == HC - 1))
                        for i in range(2):
                            nc.vector.tensor_scalar_add(
                                projT[n][:, i * 1024:(i + 1) * 1024],
                                accs[i], bias_sb[n])

                    gvaug = {}
                    for pi, n in enumerate(W_NAMES):
                        if pi + 3 < len(W_NAMES):
                            emit_wdma(W_NAMES[pi + 3])
                        emit_proj(n)
                        # vaug builds only need projT_wv; slot them after
                        # later projections to fill weight-DMA wait gaps
                        if n == "wlq":
                            gvaug[0] = build_vaug(projT["wv"][0:64])
                        elif n == "wlk1":
                            gvaug[1] = build_vaug(projT["wv"][64:128])

                # ---------- phase A2: global attention + local prep +
                # pass-1 max sweep (interleaved) ----------
                with (
                    tc.tile_pool(name="vpool", bufs=1) as vp,
                    tc.tile_pool(name="epool", bufs=4) as ep,
                    tc.tile_pool(name="opool", bufs=1) as op_,
                ):
                    out_sb = op_.tile([128, SC, 256], f32, name="out_sb")
                    st2 = local_prep(2)
                    st3 = local_prep(3)

                    f2 = make_filler(
                        [(st2, t, jp) for t in range(SC) for jp in range(2)])
                    f3 = make_filler(
                        [(st3, t, jp) for t in range(SC) for jp in range(2)])
                    pend = []
                    for u, (hh, jp) in enumerate(
                            [(h, p) for h in range(2) for p in range(2)]):
                        rs = slice(hh * 64, (hh + 1) * 64)
                        pend = attention_pair(
                            hh, projT["wk"][rs], projT["wq"][rs],
                            gvaug[hh], False, jp,
                            filler=f2 if u < 2 else f3,
                            defer_tail=(u == 3))
                        if u == 1:
                            pass1_finish(st2)
                        if u == 3:
                            pass1_finish(st3)

                    # ---------- phase B: local pass-2 attention (each
                    # unit's output transposes run as the next unit's
                    # PE filler so unit boundaries leave no PE gap) ------
                    def make_thunk_filler(thunks):
                        state = {"i": 0}

                        def filler():
                            i = state["i"]
                            state["i"] += 1
                            if i == 2 and len(thunks) > 0:
                                thunks[0]()
                            elif i == 9 and len(thunks) > 1:
                                thunks[1]()
                        return filler

                    for head, hs in ((2, st2), (3, st3)):
                        for jp in range(2):
                            pend = attention_pair(
                                head, hs["k2aug"], hs["qaug"], hs["vaug"],
                                True, jp, filler=make_thunk_filler(pend),
                                defer_tail=True, copy_on_act=False)
                    for th in pend:
                        th()

    nc.compile()
    return nc


def kernel(**inputs):
    from concourse import bass_utils

    global LAST_RESULTS
    if "nc" not in _CACHE:
        _CACHE["nc"] = _build()
    nc = _CACHE["nc"]

    inputs = dict(inputs)
    inputs["wlv"] = np.asarray(inputs["wlv1"]) + np.asarray(inputs["wlv2"])
    inputs["blv"] = np.asarray(inputs["blv1"]) + np.asarray(inputs["blv2"])
    hs = np.ascontiguousarray(np.asarray(inputs["hidden_states"], np.float32))
    am = np.ascontiguousarray(np.asarray(inputs["attention_mask"], np.float32))
    in_maps = []
    for c in range(N_CORES):
        b, g = c // 4, c % 4
        csl = slice(128 * g, 128 * (g + 1))
        m = {"hid": np.ascontiguousarray(hs[b].T), "mask": am[b, 0, 0]}
        for n in W_NAMES:
            m[n] = np.ascontiguousarray(
                np.asarray(inputs[n], np.float32)[:, csl])
            m["b" + n[1:]] = np.ascontiguousarray(
                np.asarray(inputs["b" + n[1:]], np.float32)[csl])
        in_maps.append(m)

    res = bass_utils.run_bass_kernel_spmd(
        nc, in_maps, list(range(N_CORES)),
        tmpdir=os.environ.get("BASS_TMPDIR"))
    LAST_RESULTS = res

    out = np.zeros((B, S, HID), np.float32)
    for c in range(N_CORES):
        b, g = c // 4, c % 4
        o = res.results[c]["out"]
        out[b, :, 128 * g:128 * (g + 1)] = o[:, :128]
        out[b, :, 512 + 128 * g:512 + 128 * (g + 1)] = o[:, 128:]
    return out


# revision 27
# speedup vs baseline: 1.0394x; 1.0394x over previous
"""Trainium2 Bass kernel for nn_MixedAttention.

Full inputs in, full output out. Sharding: 8 cores = 2 (batch) x 4 (head
pairs). Each core computes 2 global + 2 local heads for one batch element.

Key algebraic rewrite for the local branch:
    lscores = (lq@lk1^T)@(lk1@lk2^T) = lq @ (lk1^T@lk1) @ lk2^T
with M = lk1^T@lk1 a [64,64] matrix -- turns a 2048^3 matmul chain into
two small matmuls plus one S x S matmul (30x less PE work).

Dtype strategy: fp32 matmuls cost 4 cycles/row on the PE, float32r
(TF32-like, ~13-bit mantissa) costs 1 at free-size >= 512. Measured
rel-err budget is 2e-2 and a numerical simulation of 13-bit input
rounding through the local score chain lands at ~1.4e-3, so EVERY
matmul input here is f32r (storage is bit-identical to fp32; the PE
rounds internally). hidden/weights are declared f32r straight from
DRAM so no conversion copies exist anywhere.

Layout: scores are computed transposed st[j, i] = K_eff @ Q_eff^T so the
context matmul needs no transposed probs (lhsT = v_nat, rhs = e). v gets
an extra ones column so the softmax denominator falls out of the context
matmul for free. Global heads skip max subtraction entirely (mask folded
into the Exp bias); local heads get an exact row max from a pass-1 f32r
matmul sweep in the untransposed orientation (free-dim reduce_max), and
the -max correction rides an extra contraction row (K=65) in pass 2.

Schedule: phase A1 = input DMA (split fine-grained over 4 trigger
queues) + all 7 projections with 4-wide PSUM accumulation. Phase A2 =
global attention pair-units interleaved with local-prep and the pass-1
max sweep (hides pass-1's DVE reduces under global-attention PE work).
Phase B = local pass-2 attention. Attention works in ic-pairs: one
[128,1024] two-bank PSUM st tile per j-block, one Exp activation per
j-block, ctx accumulated in a [65,1024] two-bank PSUM tile.
"""

import math
import os
import sys

import numpy as np

sys.path.insert(0, "/opt/trn_rl_repo")

B, S, HID, HEAD = 2, 2048, 1024, 64
SC = S // 128  # 16 s-chunks of 128
HC = HID // 128  # 8 hidden chunks
N_CORES = 8
SCALE = 1.0 / math.sqrt(HEAD)

W_NAMES = ["wq", "wk", "wv", "wlq", "wlk1", "wlk2", "wlv"]

_CACHE = {}
LAST_RESULTS = None  # stash of BassKernelResults for test.py profiling


def _build():
    import concourse.mybir as mybir
    import concourse.tile as tile
    from concourse import bacc
    from concourse.masks import make_identity

    f32 = mybir.dt.float32
    f32r = mybir.dt.float32r
    bf16 = mybir.dt.bfloat16
    AF = mybir.ActivationFunctionType
    ALU = mybir.AluOpType
    AX = mybir.AxisListType

    nc = bacc.Bacc("TRN2", target_bir_lowering=False, debug=False,
                   enable_asserts=False)

    hid_d = nc.dram_tensor("hid", (HID, S), f32r, kind="ExternalInput").ap()
    mask_d = nc.dram_tensor("mask", (S,), f32, kind="ExternalInput").ap()
    w_d = {n: nc.dram_tensor(n, (128, HC, 128), f32r,
                             kind="ExternalInput").ap()
           for n in W_NAMES}
    b_d = {n: nc.dram_tensor("b" + n[1:], (128,), f32,
                             kind="ExternalInput").ap() for n in W_NAMES}
    out_d = nc.dram_tensor("out", (S, 256), f32, kind="ExternalOutput").ap()

    dma_engines = None  # set inside the TileContext

    def dma_rr(i):
        return dma_engines[i % len(dma_engines)]

    with tile.TileContext(nc) as tc:
        dma_engines = [nc.sync, nc.gpsimd, nc.scalar]
        with (
            tc.tile_pool(name="const", bufs=1) as constp,
            tc.tile_pool(name="persist", bufs=1) as pp,
            tc.tile_pool(name="vgpool", bufs=1) as vgp,
            tc.tile_pool(name="ps_mm", bufs=3, space="PSUM") as ps_mm,
            tc.tile_pool(name="ps_ctx", bufs=1, space="PSUM") as ps_ctx,
            tc.tile_pool(name="dramp", bufs=2, space="DRAM") as dramp,
        ):
            ident = constp.tile([128, 128], f32, name="ident")
            make_identity(nc, ident)
            ones_sb = constp.tile([128, SC], f32, name="ones_sb")
            nc.vector.memset(ones_sb, 1.0)
            ones_row = constp.tile([1, S], f32, name="ones_row")
            nc.vector.memset(ones_row, 1.0)
            mask_sb = constp.tile([128, SC], f32, name="mask_sb")
            nc.gpsimd.dma_start(mask_sb,
                                mask_d.rearrange("(c p) -> p c", p=128))
            bias_sb = {}
            for n in W_NAMES:
                t = constp.tile([128, 1], f32, name=f"b_{n}")
                nc.gpsimd.dma_start(t, b_d[n][:, None])
                bias_sb[n] = t

            # local-branch projections persist into phase B
            projT = {n: pp.tile([128, S],
                                f32 if n in ("wlk1", "wlv") else f32r,
                                name=f"projT_{n}")
                     for n in ["wlq", "wlk1", "wlk2", "wlv"]}

            # assigned when the A2/B pools open (the helpers below only
            # run after that)
            vp = ep = op_ = out_sb = None

            # ---------- emission helpers ----------

            def build_vaug(vT):
                # v natural [s, d] + ones column -> [128, SC, 65] bf16
                # (bf16 halves the per-ctx-matmul LDWEIGHTS stream; value
                # path tolerates the 8-bit mantissa)
                base = vT.base_partition()
                idsl = slice(base, base + 64)
                vaug = vgp.tile([128, SC, 65], bf16, tag="vaug",
                                name="vaug", bufs=4)
                nc.vector.tensor_copy(vaug[:, :, 64], ones_sb)
                for t in range(SC):
                    pt = ps_mm.tile([128, 1024], f32, tag="mm", name="ptv")
                    nc.tensor.transpose(
                        pt[:, :64], vT[:, t * 128:(t + 1) * 128],
                        ident[idsl, idsl])
                    nc.vector.tensor_copy(vaug[:, t, :64], pt[:, :64])
                return vaug

            def attention_pair(head, kT, qT, vaug, is_local, jp,
                               filler=None, defer_tail=False,
                               copy_on_act=True):
                # pair-unit: 2 i-column blocks of 512; st -> exp -> ctx
                # (+denominators via the ones column), then transpose back
                # and divide by the sums. `filler` emits ACT-independent PE
                # work each jc so the tensor engine never starves while the
                # Exp runs (starvation resets the PE's DVFS ramp).
                csl = slice(head * 64, (head + 1) * 64)
                i0 = jp * 1024
                ctx = ps_ctx.tile([65, 1024], f32, tag="ctx", name="ctx")

                def ctx_mm(jc, e):
                    nc.tensor.matmul(ctx[:, 0:512], lhsT=vaug[:, jc],
                                     rhs=e[:, 0:512],
                                     start=(jc == 0), stop=(jc == SC - 1))
                    nc.tensor.matmul(ctx[:, 512:1024], lhsT=vaug[:, jc],
                                     rhs=e[:, 512:1024],
                                     start=(jc == 0), stop=(jc == SC - 1))

                # one-jc software pipeline lag: the ctx matmuls for jc are
                # emitted after st/exp of jc+1, so the in-order PE queue
                # never waits on the Exp of the tile it just produced
                prev = None
                for jc in range(SC):
                    jsl = slice(jc * 128, (jc + 1) * 128)
                    st = ps_mm.tile([128, 1024], f32, tag="mm", name="st")
                    nc.tensor.matmul(st[:, 0:512], lhsT=kT[:, jsl],
                                     rhs=qT[:, i0:i0 + 512],
                                     start=True, stop=True)
                    nc.tensor.matmul(st[:, 512:1024], lhsT=kT[:, jsl],
                                     rhs=qT[:, i0 + 512:i0 + 1024],
                                     start=True, stop=True)
                    e = ep.tile([128, 1024], bf16, tag="e", name="e")
                    bias = 0.0 if is_local else mask_sb[:, jc:jc + 1]
                    nc.scalar.activation(e, st, AF.Exp, bias=bias,
                                         scale=SCALE)
                    if filler is not None:
                        filler()
                    if prev is not None:
                        ctx_mm(*prev)
                    prev = (jc, e)
                ctx_mm(*prev)
                ctx_sbc = vp.tile([65, 1024], f32, tag="ctx_sbc",
                                  name="ctx_sbc", bufs=2)
                # the PSUM->SBUF ctx copy rides ACT when DVE is the busier
                # engine (phase A2) and DVE when ACT paces the phase (B)
                if copy_on_act:
                    nc.scalar.copy(ctx_sbc, ctx)
                else:
                    nc.vector.tensor_copy(ctx_sbc, ctx)

                def make_tg(tg):
                    def run():
                        pts = ps_mm.tile([128, 1024], f32, tag="mm",
                                         name="pts")
                        rec = vp.tile([128, 4], f32, tag="rec", name="rec",
                                      bufs=2)
                        for q in range(4):
                            tt = tg * 4 + q
                            nc.tensor.transpose(
                                pts[:, q * 256:q * 256 + 65],
                                ctx_sbc[:, tt * 128:(tt + 1) * 128],
                                ident[:65, :65])
                            nc.vector.reciprocal(
                                rec[:, q:q + 1],
                                pts[:, q * 256 + 64:q * 256 + 65])
                        for q in range(4):
                            tt = tg * 4 + q
                            t_abs = jp * 8 + tt
                            nc.vector.tensor_scalar_mul(
                                out_sb[:, t_abs, csl],
                                pts[:, q * 256:q * 256 + 64],
                                rec[:, q:q + 1])
                        eng = nc.gpsimd if tg == 0 else nc.sync
                        eng.dma_start(
                            out_d.rearrange("(t p) c -> p t c", p=128)[
                                :, jp * 8 + tg * 4:jp * 8 + tg * 4 + 4, csl],
                            out_sb[:, jp * 8 + tg * 4:jp * 8 + tg * 4 + 4,
                                   csl])
                    return run

                tail = [make_tg(0), make_tg(1)]
                if defer_tail:
                    return tail
                for th in tail:
                    th()
                return []

            def local_prep(head):
                hh = head % 2
                rs = slice(hh * 64, (hh + 1) * 64)
                idsl = slice(rs.start, rs.start + 64)
                if hh == 0:
                    lqT = projT["wlq"][rs]
                else:
                    # matmul operands must share a base partition and the
                    # PSUM dst must sit at partition 0, so head hh=1's lq
                    # is staged down to base 0 (ACT handles the shift)
                    lqT = vp.tile([64, S], f32r, tag="lqT", name="lqT",
                                  bufs=1)
                    nc.scalar.copy(lqT, projT["wlq"][rs])
                lk1T = projT["wlk1"][rs]

                # lk1 natural [s, d] via transposes (identity block at the
                # source base partition avoids any staging copy)
                lk1nat = vp.tile([128, SC, 64], f32r, tag="lk1nat",
                                 name="lk1nat", bufs=2)
                for t in range(SC):
                    pt = ps_mm.tile([128, 1024], f32, tag="mm", name="ptk")
                    nc.tensor.transpose(
                        pt[:, :64], lk1T[:, t * 128:(t + 1) * 128],
                        ident[idsl, idsl])
                    nc.vector.tensor_copy(lk1nat[:, t], pt[:, :64])
                # M = lk1^T @ lk1 [64, 64] (symmetric)
                mps = ps_mm.tile([128, 1024], f32, tag="mm", name="mps")
                for t in range(SC):
                    nc.tensor.matmul(mps[:64, :64], lhsT=lk1nat[:, t],
                                     rhs=lk1nat[:, t],
                                     start=(t == 0), stop=(t == SC - 1))
                m_sb = vp.tile([64, 64], f32r, tag="m_sb", name="m_sb",
                               bufs=2)
                nc.vector.tensor_copy(m_sb, mps[:64, :64])
                # qaug rows 0:64 = (lq @ M)^T = M @ lq^T (M symmetric);
                # row 64 filled later with -max
                qaug = vp.tile([65, S], f32r, tag="qaug", name="qaug",
                               bufs=2)
                for half in range(2):
                    mm = ps_mm.tile([128, 1024], f32, tag="mm", name="mm")
                    for ic in range(2):
                        icg = half * 2 + ic
                        nc.tensor.matmul(
                            mm[:64, ic * 512:(ic + 1) * 512], lhsT=m_sb,
                            rhs=lqT[:, icg * 512:(icg + 1) * 512],
                            start=True, stop=True)
                    nc.vector.tensor_copy(
                        qaug[:64, half * 1024:(half + 1) * 1024], mm[:64])
                # k2aug: rows 0:64 = lk2^T, row 64 = ones
                k2aug = vp.tile([65, S], f32r, tag="k2aug", name="k2aug",
                                bufs=2)
                nc.scalar.copy(k2aug[:64, :], projT["wlk2"][rs])
                nc.vector.tensor_copy(k2aug[64:65, :], ones_row)
                # bf16 shadows for the pass-1 max sweep: 16-bit moving data
                # streams at 1 cyc/col on the PE vs 2 for f32r, and the max
                # only needs ~1 unit of absolute accuracy out of +/-80
                qaug_bf = vp.tile([64, S], bf16, tag="qaug_bf",
                                  name="qaug_bf", bufs=2)
                nc.vector.tensor_copy(qaug_bf, qaug[:64])
                k2aug_bf = vp.tile([64, S], bf16, tag="k2aug_bf",
                                   name="k2aug_bf", bufs=2)
                nc.vector.tensor_copy(k2aug_bf, k2aug[:64])
                vaug = build_vaug(projT["wlv"][rs])
                pmax = vp.tile([128, SC, 2], f32, tag="pmax", name="pmax",
                               bufs=2)
                return dict(qaug=qaug, k2aug=k2aug, vaug=vaug, pmax=pmax,
                            qaug_bf=qaug_bf, k2aug_bf=k2aug_bf)

            def pass1_pair(hs, t, jp):
                # one (t, jp) unit of the pass-1 max sweep: raw scores in
                # the untransposed orientation, row max via free-dim reduce
                qaug, k2aug, pmax = hs["qaug_bf"], hs["k2aug_bf"], \
                    hs["pmax"]
                tsl = slice(t * 128, (t + 1) * 128)
                st = ps_mm.tile([128, 1024], f32, tag="mm", name="st1")
                for j2 in range(2):
                    j0 = jp * 1024 + j2 * 512
                    nc.tensor.matmul(st[:, j2 * 512:(j2 + 1) * 512],
                                     lhsT=qaug[:, tsl],
                                     rhs=k2aug[:, j0:j0 + 512],
                                     start=True, stop=True)
                nc.vector.tensor_reduce(pmax[:, t, jp:jp + 1], st,
                                        axis=AX.X, op=ALU.max)

            def make_filler(units):
                it = iter(units)

                def filler():
                    u = next(it, None)
                    if u is not None:
                        pass1_pair(*u)
                return filler

            def pass1_finish(hs):
                # combine pair maxes, negate, and route [128, SC] -> [1, S]
                # via a DRAM roundtrip into qaug row 64
                maxneg = vp.tile([128, SC], f32r, tag="maxneg",
                                 name="maxneg", bufs=2)
                nc.vector.tensor_reduce(maxneg, hs["pmax"], axis=AX.X,
                                        op=ALU.max, negate=True)
                mscr = dramp.tile([S], f32r, tag="mscr", name="mscr")
                nc.sync.dma_start(
                    mscr.rearrange("(t p) -> p t", p=128), maxneg)
                nc.sync.dma_start(hs["qaug"][64:65, :], mscr[None, :])

            # ---------- phase A1: hidden^T + all 7 projections ----------
            with tc.tile_pool(name="pp_g", bufs=1) as pp_g:
                for n in ["wq", "wk", "wv"]:
                    projT[n] = pp_g.tile([128, S],
                                         f32 if n == "wv" else bf16,
                                         name=f"projT_{n}")

                with (
                    tc.tile_pool(name="hidT", bufs=1) as hp,
                    tc.tile_pool(name="io", bufs=4) as iop,
                ):
                    hidT = hp.tile([128, HC, S], f32r, name="hidT")
                    hidT_bf = hp.tile([128, HC, S], bf16, name="hidT_bf")
                    hid_r = hid_d.rearrange("(c p) s -> p c s", p=128)
                    dmai = 0
                    wsbs = {}

                    def emit_wdma(n):
                        nonlocal dmai
                        wsb = iop.tile([128, HC, 128], f32r, tag="w",
                                       name=f"w_{n}")
                        dma_rr(dmai).dma_start(wsb, w_d[n])
                        dmai += 1
                        if n in ("wq", "wk"):
                            wbf = iop.tile([128, HC, 128], bf16, tag="wbf",
                                           name=f"wbf_{n}", bufs=2)
                            nc.vector.tensor_copy(wbf, wsb)
                            wsbs[n] = wbf
                        else:
                            wsbs[n] = wsb

                    # bf16-path projections (wq, wk) go LAST so the hid
                    # bf16 casts are fully pipelined by the time they run
                    PROJ_ORDER = ["wv", "wlq", "wlk1", "wlk2", "wlv",
                                  "wq", "wk"]
                    for n in PROJ_ORDER[:3]:
                        emit_wdma(n)
                    for hc in range(HC):
                        for h2 in range(2):
                            ssl = slice(h2 * 1024, (h2 + 1) * 1024)
                            dma_rr(dmai).dma_start(hidT[:, hc, ssl],
                                                   hid_r[:, hc, ssl])
                            dmai += 1
                        nc.vector.tensor_copy(hidT_bf[:, hc], hidT[:, hc])

                    def emit_proj(n):
                        bf = n in ("wq", "wk")
                        ht = hidT_bf if bf else hidT
                        accs = [ps_mm.tile([128, 1024], f32, tag="mm",
                                           name=f"acc{i}") for i in range(2)]
                        for hc in range(HC):
                            for ic in range(4):
                                nc.tensor.matmul(
                                    accs[ic // 2][:, (ic % 2) * 512:
                                                  (ic % 2 + 1) * 512],
                                    lhsT=wsbs[n][:, hc],
                                    rhs=ht[:, hc, ic * 512:(ic + 1) * 512],
                                    start=(hc == 0), stop=(hc == HC - 1))
                        for i in range(2):
                            nc.vector.tensor_scalar_add(
                                projT[n][:, i * 1024:(i + 1) * 1024],
                                accs[i], bias_sb[n])

                    gvaug = {}
                    for pi, n in enumerate(PROJ_ORDER):
                        if pi + 3 < len(PROJ_ORDER):
                            emit_wdma(PROJ_ORDER[pi + 3])
                        emit_proj(n)
                        # vaug builds only need projT_wv (first); slot them
                        # after later projections to fill weight-DMA gaps
                        if n == "wlq":
                            gvaug[0] = build_vaug(projT["wv"][0:64])
                        elif n == "wlk1":
                            gvaug[1] = build_vaug(projT["wv"][64:128])

                # ---------- phase A2: global attention + local prep +
                # pass-1 max sweep (interleaved) ----------
                with (
                    tc.tile_pool(name="vpool", bufs=1) as vp,
                    tc.tile_pool(name="epool", bufs=4) as ep,
                    tc.tile_pool(name="opool", bufs=1) as op_,
                ):
                    out_sb = op_.tile([128, SC, 256], f32, name="out_sb")
                    st2 = local_prep(2)
                    st3 = local_prep(3)

                    f2 = make_filler(
                        [(st2, t, jp) for t in range(SC) for jp in range(2)])
                    f3 = make_filler(
                        [(st3, t, jp) for t in range(SC) for jp in range(2)])
                    pend = []
                    for u, (hh, jp) in enumerate(
                            [(h, p) for h in range(2) for p in range(2)]):
                        rs = slice(hh * 64, (hh + 1) * 64)
                        pend = attention_pair(
                            hh, projT["wk"][rs], projT["wq"][rs],
                            gvaug[hh], False, jp,
                            filler=f2 if u < 2 else f3,
                            defer_tail=(u == 3))
                        if u == 1:
                            pass1_finish(st2)
                        if u == 3:
                            pass1_finish(st3)

                    # ---------- phase B: local pass-2 attention (each
                    # unit's output transposes run as the next unit's
                    # PE filler so unit boundaries leave no PE gap) ------
                    def make_thunk_filler(thunks):
                        state = {"i": 0}

                        def filler():
                            i = state["i"]
                            state["i"] += 1
                            if i == 2 and len(thunks) > 0:
                                thunks[0]()
                            elif i == 9 and len(thunks) > 1:
                                thunks[1]()
                        return filler

                    for head, hs in ((2, st2), (3, st3)):
                        for jp in range(2):
                            pend = attention_pair(
                                head, hs["k2aug"], hs["qaug"], hs["vaug"],
                                True, jp, filler=make_thunk_filler(pend),
                                defer_tail=True, copy_on_act=False)
                    for th in pend:
                        th()

    nc.compile()
    return nc


def kernel(**inputs):
    from concourse import bass_utils

    global LAST_RESULTS
    if "nc" not in _CACHE:
        _CACHE["nc"] = _build()
    nc = _CACHE["nc"]

    inputs = dict(inputs)
    inputs["wlv"] = np.asarray(inputs["wlv1"]) + np.asarray(inputs["wlv2"])
    inputs["blv"] = np.asarray(inputs["blv1"]) + np.asarray(inputs["blv2"])
    hs = np.ascontiguousarray(np.asarray(inputs["hidden_states"], np.float32))
    am = np.ascontiguousarray(np.asarray(inputs["attention_mask"], np.float32))
    in_maps = []
    for c in range(N_CORES):
        b, g = c // 4, c % 4
        csl = slice(128 * g, 128 * (g + 1))
        m = {"hid": np.ascontiguousarray(hs[b].T), "mask": am[b, 0, 0]}
        for n in W_NAMES:
            w = np.asarray(inputs[n], np.float32)[:, csl]
            m[n] = np.ascontiguousarray(
                w.reshape(HC, 128, 128).transpose(1, 0, 2))
            m["b" + n[1:]] = np.ascontiguousarray(
                np.asarray(inputs["b" + n[1:]], np.float32)[csl])
        in_maps.append(m)

    res = bass_utils.run_bass_kernel_spmd(
        nc, in_maps, list(range(N_CORES)),
        tmpdir=os.environ.get("BASS_TMPDIR"))
    LAST_RESULTS = res

    out = np.zeros((B, S, HID), np.float32)
    for c in range(N_CORES):
        b, g = c // 4, c % 4
        o = res.results[c]["out"]
        out[b, :, 128 * g:128 * (g + 1)] = o[:, :128]
        out[b, :, 512 + 128 * g:512 + 128 * (g + 1)] = o[:, 128:]
    return out
